# revision 21
# baseline (speedup 1.0000x reference)
"""Trainium2 Bass kernel for hyperedge segment-reduce (Maxmin) + MLP decoder.

Computation (matches the reference nn.Module):
    feats = v_feat[node_ids]                        # [E, D] gather
    emb   = segment_max(feats) - segment_min(feats) # [NH, D], segments = groups of 16
    out   = sigmoid(relu(relu(emb@W1+b1)@W2+b2)@W3+b3)   # [NH, 1]

Sharding: hyperedges are split evenly across 8 NeuronCores.  Instead of
replicating the full v_feat, each core receives per-chunk COMPACTED node
tables: for each chunk of hyperedge-blocks the host ships
vfeat[sorted unique nodes referenced by the chunk] plus int16 positions
into that table.  unique <= 14336 < 32768, so the bulk `dma_gather`
custom SWDGE instruction (int16 indices) applies.  Four SWDGE queues run
descriptor generation on all four Q7 core pairs concurrently (the Q7
descriptor-generation rate of ~8ns/row is the kernel's bottleneck).

Chunk sizes are tapered (small first and last chunk) to shorten pipeline
fill and drain.

Per-core device program, per chunk:
  - 4x dma_gather (one per SWDGE queue): dest[p, c] = table[idx[c*128+p]];
    column c = b*16+m holds member m of hyperedge (boff+b)*128 + p.
  - DVE halving-tree (4 levels of tensor_tensor max / min over the whole
    chunk) computes segment max and min; emb = max - min.
  - PE transpose per 128x128 block -> emb^T (feature-on-partition layout).
  - 3-layer MLP on PE (matmuls) + ACT (bias+relu / bias+sigmoid).
"""

import os
import numpy as np

import concourse.bass as bass
import concourse.mybir as mybir
from concourse import bacc, tile, bass_utils
from concourse.masks import make_identity

# ---------------------------------------------------------------- constants
N_NODES = 100000
D = 128
NH = 50000
G = 16
E = NH * G
NCORES = 8
H_CORE = NH // NCORES           # 6250 hyperedges per core
BLKS = 49                       # ceil(6250/128)
HPAD = BLKS * 128               # 6272 (padded hyperedges per core)

# blocks per gather chunk (sum = 49); tapered ends shrink fill/drain.
CHUNKS = [2, 11, 11, 11, 11, 3]
NCHUNK = len(CHUNKS)
BOFF = [0]
for _b in CHUNKS:
    BOFF.append(BOFF[-1] + _b)
assert BOFF[-1] == BLKS
MAXB = max(CHUNKS)              # 7
ROWS = MAXB * G                 # 112 gather columns (max, per partition)
IDXCOLS = BLKS * 2048 // 16     # 6272 idx-tile columns total

# dtype knobs ("f32" or "bf16").
GATHER_DT = os.environ.get("KERNEL_GATHER_DT", "bf16")
MLP_DT = os.environ.get("KERNEL_MLP_DT", "f32")
MC_BUFS = int(os.environ.get("KERNEL_MC_BUFS", "2"))
MX_BUFS = int(os.environ.get("KERNEL_MX_BUFS", "1"))
# SWDGE descriptor-ring carveout; 32KB = 512 descs/ring so a 5632-idx
# sub-gather (353 descs/engine) fits in one ring entry.
DMA_SCRATCH = int(os.environ.get("KERNEL_DMA_SCRATCH", "32768"))
GSUB = int(os.environ.get("KERNEL_GSUB", "4"))    # sub-gathers per chunk
NQUEUES = int(os.environ.get("KERNEL_NQUEUES", "4"))  # SWDGE queues (Q7 pairs)

_DT = {"f32": mybir.dt.float32, "bf16": mybir.dt.bfloat16}

f32 = mybir.dt.float32
i16 = mybir.dt.int16


def _splits(nb):
    """MLP free-dim splits (<=512 cols each) for a chunk of nb blocks."""
    cols = nb * 128
    out = []
    off = 0
    while cols > 0:
        w = min(512, cols)
        out.append((off, w))
        off += w
        cols -= w
    return out


# ---------------------------------------------------------------- device IR
def build_module():
    gdt = _DT[GATHER_DT]
    mdt = _DT[MLP_DT]

    nc = bacc.Bacc(
        "TRN2",
        target_bir_lowering=False,
        debug=False,
        enable_asserts=False,
        num_devices=NCORES,
        num_swdge_queues=NQUEUES,
        dynamic_dma_scratch_size=DMA_SCRATCH,
    )

    tab = nc.dram_tensor("tab", [BLKS * 2048, D], gdt, kind="ExternalInput")
    idx = nc.dram_tensor("idx", [128, IDXCOLS], i16, kind="ExternalInput")
    w1 = nc.dram_tensor("w1", [128, 256], mdt, kind="ExternalInput")
    b1 = nc.dram_tensor("b1", [128, 2], f32, kind="ExternalInput")
    w2 = nc.dram_tensor("w2", [128, 256], mdt, kind="ExternalInput")
    b2 = nc.dram_tensor("b2", [128, 1], f32, kind="ExternalInput")
    w3 = nc.dram_tensor("w3", [128, 1], mdt, kind="ExternalInput")
    b3 = nc.dram_tensor("b3", [1, 1], f32, kind="ExternalInput")
    idn = nc.dram_tensor("ident", [128, 128], mdt, kind="ExternalInput")
    out = nc.dram_tensor("out", [HPAD], f32, kind="ExternalOutput")

    out2d = out.ap().rearrange("(a b) -> a b", a=1)  # [1, HPAD]

    with tile.TileContext(nc) as tc:
        with (
            tc.tile_pool(name="const", bufs=1) as cp,
            tc.tile_pool(name="mem", bufs=2) as mp,
            tc.tile_pool(name="scr", bufs=2) as sp,
            tc.tile_pool(name="mlp", bufs=2) as lp,
            tc.tile_pool(name="pst", bufs=2, space="PSUM") as pt,
            tc.tile_pool(name="psm", bufs=1, space="PSUM") as pm,
        ):
            ident = cp.tile([128, 128], mdt)
            w1_t = cp.tile([128, 256], mdt)
            b1_t = cp.tile([128, 2], f32)
            w2_t = cp.tile([128, 256], mdt)
            b2_t = cp.tile([128, 1], f32)
            w3_t = cp.tile([128, 1], mdt)
            b3_t = cp.tile([1, 1], f32)

            nc.sync.dma_start(out=ident[:], in_=idn.ap())
            nc.sync.dma_start(out=w1_t[:], in_=w1.ap())
            nc.sync.dma_start(out=b1_t[:], in_=b1.ap())
            nc.sync.dma_start(out=w2_t[:], in_=w2.ap())
            nc.sync.dma_start(out=b2_t[:], in_=b2.ap())
            nc.sync.dma_start(out=w3_t[:], in_=w3.ap())
            nc.sync.dma_start(out=b3_t[:], in_=b3.ap())

            for ch in range(NCHUNK):
                nb = CHUNKS[ch]
                memb = nb * 2048              # member slots this chunk
                trow0 = BOFF[ch] * 2048       # table row base
                icol0 = BOFF[ch] * 128        # idx-tile column base

                # ---- per-chunk idx load (small, ahead of the gather) ----
                ib = mp.tile([128, MAXB * 128], i16, tag="ib", bufs=3)
                nc.sync.dma_start(
                    out=ib[:, :memb // 16],
                    in_=idx.ap()[:, icol0:icol0 + memb // 16])

                # ---- gather the chunk with GSUB dma_gathers (4 queues) ----
                Mc = mp.tile([128, ROWS * D], gdt, tag="mc", bufs=MC_BUFS)
                M3 = Mc[:, :nb * G * D].rearrange("p (c d) -> p c d", d=D)
                ni = memb // GSUB             # idx per sub-gather
                nw = ni // 16                 # idx-tile columns per sub
                nd = ni // 128                # dest columns per sub
                for g in range(GSUB):
                    nc.gpsimd.dma_gather(
                        out_ap=M3[:, g * nd:(g + 1) * nd, :],
                        in_ap=tab.ap()[trow0:trow0 + memb, :],
                        idxs_ap=ib[:, g * nw:(g + 1) * nw],
                        num_idxs=ni,
                        num_idxs_reg=ni,
                        elem_size=D,
                        single_packet=False,
                        queue_num=g % NQUEUES,
                    )

                M4 = Mc[:, :nb * G * D].rearrange("p (b m d) -> p b m d",
                                                  m=G, d=D)

                # ---- segment max (tree into scratch, whole chunk) ----
                mx = sp.tile([128, MAXB * 8 * D], gdt, tag="mx", bufs=MX_BUFS)
                mx4 = mx[:, :nb * 8 * D].rearrange("p (b m d) -> p b m d",
                                                   m=8, d=D)
                nc.vector.tensor_tensor(
                    out=mx4, in0=M4[:, :, 0:8, :], in1=M4[:, :, 8:16, :],
                    op=mybir.AluOpType.max)
                nc.vector.tensor_tensor(
                    out=mx4[:, :, 0:4, :], in0=mx4[:, :, 0:4, :],
                    in1=mx4[:, :, 4:8, :], op=mybir.AluOpType.max)
                nc.vector.tensor_tensor(
                    out=mx4[:, :, 0:2, :], in0=mx4[:, :, 0:2, :],
                    in1=mx4[:, :, 2:4, :], op=mybir.AluOpType.max)
                nc.vector.tensor_tensor(
                    out=mx4[:, :, 0:1, :], in0=mx4[:, :, 0:1, :],
                    in1=mx4[:, :, 1:2, :], op=mybir.AluOpType.max)

                # ---- segment min (tree in place inside Mc) ----
                nc.vector.tensor_tensor(
                    out=M4[:, :, 0:8, :], in0=M4[:, :, 0:8, :],
                    in1=M4[:, :, 8:16, :], op=mybir.AluOpType.min)
                nc.vector.tensor_tensor(
                    out=M4[:, :, 0:4, :], in0=M4[:, :, 0:4, :],
                    in1=M4[:, :, 4:8, :], op=mybir.AluOpType.min)
                nc.vector.tensor_tensor(
                    out=M4[:, :, 0:2, :], in0=M4[:, :, 0:2, :],
                    in1=M4[:, :, 2:4, :], op=mybir.AluOpType.min)
                nc.vector.tensor_tensor(
                    out=M4[:, :, 0:1, :], in0=M4[:, :, 0:1, :],
                    in1=M4[:, :, 1:2, :], op=mybir.AluOpType.min)

                # ---- emb = max - min  (hedge-on-partition layout) ----
                emb = sp.tile([128, MAXB * D], mdt, tag="emb", bufs=2)
                emb4 = emb[:, :nb * D].rearrange("p (b m d) -> p b m d",
                                                 m=1, d=D)
                nc.vector.tensor_tensor(
                    out=emb4, in0=mx4[:, :, 0:1, :], in1=M4[:, :, 0:1, :],
                    op=mybir.AluOpType.subtract)

                # ---- transpose each 128x128 block into embT ----
                embT = lp.tile([128, MAXB * 128], mdt, tag="embT")
                for b in range(nb):
                    ptile = pt.tile([128, 128], mdt, tag="ptr")
                    nc.tensor.transpose(
                        out=ptile[:], in_=emb[:, b * D:(b + 1) * D],
                        identity=ident[:])
                    nc.scalar.copy(
                        out=embT[:, b * 128:(b + 1) * 128], in_=ptile[:])

                # ---- MLP on the chunk ----
                for (so, W) in _splits(nb):
                    ns = slice(so, so + W)
                    h1 = lp.tile([128, 2 * 512], mdt, tag="h1")
                    p2 = pm.tile([128, 512], f32, tag="p2")
                    for o in range(2):
                        p1 = pm.tile([128, 512], f32, tag=f"p1{o}")
                        nc.tensor.matmul(
                            out=p1[:, :W], lhsT=w1_t[:, o * 128:(o + 1) * 128],
                            rhs=embT[:, ns], start=True, stop=True)
                        nc.scalar.activation(
                            out=h1[:, o * 512:o * 512 + W], in_=p1[:, :W],
                            func=mybir.ActivationFunctionType.Relu,
                            bias=b1_t[:, o:o + 1])
                    nc.tensor.matmul(
                        out=p2[:, :W], lhsT=w2_t[:, 0:128], rhs=h1[:, 0:W],
                        start=True, stop=False)
                    nc.tensor.matmul(
                        out=p2[:, :W], lhsT=w2_t[:, 128:256],
                        rhs=h1[:, 512:512 + W], start=False, stop=True)
                    h2 = lp.tile([128, 512], mdt, tag="h2")
                    nc.scalar.activation(
                        out=h2[:, :W], in_=p2[:, :W],
                        func=mybir.ActivationFunctionType.Relu,
                        bias=b2_t[:, 0:1])
                    p3 = pm.tile([1, 512], f32, tag="p3")
                    nc.tensor.matmul(
                        out=p3[:, :W], lhsT=w3_t[:, 0:1], rhs=h2[:, :W],
                        start=True, stop=True)
                    osb = lp.tile([1, 512], f32, tag="osb")
                    nc.scalar.activation(
                        out=osb[:, :W], in_=p3[:, :W],
                        func=mybir.ActivationFunctionType.Sigmoid,
                        bias=b3_t[:, 0:1])
                    base = BOFF[ch] * 128 + so
                    nc.sync.dma_start(
                        out=out2d[0:1, base:base + W], in_=osb[:, :W])

    nc.compile()
    return nc


# ---------------------------------------------------------------- host prep
def _np_dt(name):
    if name == "f32":
        return np.float32
    import ml_dtypes
    return ml_dtypes.bfloat16


def prepare_in_maps(v_feat, W1, b1, W2, b2, W3, b3, node_ids):
    gnp = _np_dt(GATHER_DT)
    mnp = _np_dt(MLP_DT)

    vfeat_h = np.ascontiguousarray(np.asarray(v_feat, np.float32)).astype(gnp)
    w1_h = np.asarray(W1, np.float32).astype(mnp)                     # [128,256]
    b1_h = np.ascontiguousarray(np.asarray(b1, np.float32).reshape(2, 128).T)
    w2_h = np.concatenate(
        [np.asarray(W2, np.float32)[0:128, :], np.asarray(W2, np.float32)[128:256, :]],
        axis=1).astype(mnp)                                            # [128,256]
    b2_h = np.asarray(b2, np.float32).reshape(128, 1)
    w3_h = np.asarray(W3, np.float32).astype(mnp)                      # [128,1]
    b3_h = np.asarray(b3, np.float32).reshape(1, 1)

    nid = np.asarray(node_ids).astype(np.int64)                        # [E]

    in_maps = []
    for c in range(NCORES):
        tab_core = np.zeros((BLKS * 2048, D), gnp)
        idx_core = np.zeros((128, IDXCOLS), np.int16)
        for ch in range(NCHUNK):
            nb = CHUNKS[ch]
            memb = nb * 2048
            # hedge h_local = (BOFF[ch]+b)*128 + p (clamped), member m
            hl = ((BOFF[ch] + np.arange(nb))[:, None] * 128
                  + np.arange(128)[None, :])                           # [b,p]
            hglob = c * H_CORE + np.minimum(hl, H_CORE - 1)
            e = hglob[..., None] * G + np.arange(G)                    # [b,p,m]
            ids = nid[e]
            uniq, inv = np.unique(ids, return_inverse=True)
            tab_core[BOFF[ch] * 2048:BOFF[ch] * 2048 + len(uniq)] = vfeat_h[uniq]
            inv = inv.reshape(nb, 128, G).astype(np.int16)             # [b,p,m]
            # list position i = c_col*128 + p,  c_col = b*16 + m
            lst = inv.transpose(0, 2, 1).reshape(memb)                 # [(b,m),p]
            wrapped = lst.reshape(memb // 16, 16).T                    # [16, memb/16]
            idx_core[:, BOFF[ch] * 128:BOFF[ch] * 128 + memb // 16] = (
                np.tile(wrapped, (8, 1)))
        in_maps.append({
            "ident": np.eye(128, dtype=mnp),
            "tab": tab_core,
            "idx": idx_core,
            "w1": w1_h, "b1": b1_h,
            "w2": w2_h, "b2": b2_h,
            "w3": w3_h, "b3": b3_h,
        })
    return in_maps


def assemble_output(results):
    """results: list (per core) of {'out': [HPAD] f32} -> [NH, 1] f32."""
    outs = []
    for c in range(NCORES):
        o = np.asarray(results[c]["out"], np.float32).reshape(HPAD)
        outs.append(o[:H_CORE])
    return np.concatenate(outs).reshape(NH, 1)


# ---------------------------------------------------------------- entry
_CACHED_NC = None
LAST_RESULTS = None


def _ensure_ntff_hook():
    """The image's antenv lacks axon_hooks; if tracing is ever requested
    (e.g. BASS_TRACE in the environment), bass_utils would ImportError.
    Provide a stub so the run degrades gracefully instead of crashing."""
    import sys
    import types
    try:
        import antenv.axon_hooks  # noqa: F401
        return
    except ImportError:
        pass
    try:
        hook = None
        try:
            from trn_agent_boot.trn_boot import _ntff_profile_via_ctypes
            hook = _ntff_profile_via_ctypes("/opt/axon/libaxon_pjrt.so")
        except Exception:
            hook = None
        mod = types.ModuleType("antenv.axon_hooks")
        mod._hook = hook
        mod.get_axon_ntff_profile_hook = lambda: mod._hook
        mod.set_axon_ntff_profile_hook = lambda h: setattr(mod, "_hook", h)
        import antenv
        antenv.axon_hooks = mod
        sys.modules["antenv.axon_hooks"] = mod
    except Exception:
        pass


def _numpy_fallback(v_feat, W1, b1, W2, b2, W3, b3, node_ids, segment_ids):
    """General (slow, host) path for non-uniform segments; never taken for
    the reference's setup_inputs, which always emits repeat(arange(NH), 16)."""
    v = np.asarray(v_feat, np.float32)
    feats = v[np.asarray(node_ids).astype(np.int64)]
    seg = np.asarray(segment_ids).astype(np.int64)
    mx = np.full((NH, D), -np.inf, np.float32)
    mn = np.full((NH, D), np.inf, np.float32)
    np.maximum.at(mx, seg, feats)
    np.minimum.at(mn, seg, feats)
    emb = mx - mn
    h = np.maximum(emb @ np.asarray(W1, np.float32) + np.asarray(b1, np.float32), 0)
    h = np.maximum(h @ np.asarray(W2, np.float32) + np.asarray(b2, np.float32), 0)
    z = h @ np.asarray(W3, np.float32) + np.asarray(b3, np.float32)
    return (1.0 / (1.0 + np.exp(-z))).astype(np.float32)


def kernel(v_feat, W1, b1, W2, b2, W3, b3, node_ids, segment_ids):
    global _CACHED_NC, LAST_RESULTS

    seg = np.asarray(segment_ids)
    if seg.shape != (E,) or not np.array_equal(
            seg[::G], np.arange(NH, dtype=seg.dtype)) or not np.array_equal(
            seg, np.repeat(seg[::G], G)):
        return _numpy_fallback(v_feat, W1, b1, W2, b2, W3, b3,
                               node_ids, segment_ids)

    in_maps = prepare_in_maps(v_feat, W1, b1, W2, b2, W3, b3, node_ids)

    _ensure_ntff_hook()
    if _CACHED_NC is None:
        _CACHED_NC = build_module()
    nc = _CACHED_NC

    res = bass_utils.run_bass_kernel_spmd(
        nc, in_maps, core_ids=list(range(NCORES)))
    LAST_RESULTS = res
    return assemble_output(res.results)


# revision 22
# speedup vs baseline: 1.0938x; 1.0938x over previous
"""Trainium2 Bass kernel for hyperedge segment-reduce (Maxmin) + MLP decoder.

Computation (matches the reference nn.Module):
    feats = v_feat[node_ids]                        # [E, D] gather
    emb   = segment_max(feats) - segment_min(feats) # [NH, D], segments = groups of 16
    out   = sigmoid(relu(relu(emb@W1+b1)@W2+b2)@W3+b3)   # [NH, 1]

Sharding: hyperedges are split evenly across 8 NeuronCores.  Instead of
replicating the full v_feat, each core receives per-chunk COMPACTED node
tables: for each chunk of hyperedge-blocks the host ships
vfeat[sorted unique nodes referenced by the chunk] plus int16 positions
into that table.  unique <= 14336 < 32768, so the bulk `dma_gather`
custom SWDGE instruction (int16 indices) applies.  Four SWDGE queues run
descriptor generation on all four Q7 core pairs concurrently (the Q7
descriptor-generation rate of ~8ns/row is the kernel's bottleneck).

Chunk sizes are tapered (small first and last chunk) to shorten pipeline
fill and drain.

Per-core device program, per chunk:
  - 4x dma_gather (one per SWDGE queue): dest[p, c] = table[idx[c*128+p]];
    column c = b*16+m holds member m of hyperedge (boff+b)*128 + p.
  - DVE halving-tree (4 levels of tensor_tensor max / min over the whole
    chunk) computes segment max and min; emb = max - min.
  - PE transpose per 128x128 block -> emb^T (feature-on-partition layout).
  - 3-layer MLP on PE (matmuls) + ACT (bias+relu / bias+sigmoid).
"""

import os
import numpy as np

import concourse.bass as bass
import concourse.mybir as mybir
from concourse import bacc, tile, bass_utils
from concourse.masks import make_identity

# ---------------------------------------------------------------- constants
N_NODES = 100000
D = 128
NH = 50000
G = 16
E = NH * G
NCORES = 8
H_CORE = NH // NCORES           # 6250 hyperedges per core
BLKS = 49                       # ceil(6250/128)
HPAD = BLKS * 128               # 6272 (padded hyperedges per core)

# blocks per gather chunk (sum = 49); tapered ends shrink fill/drain.
CHUNKS = [2, 7, 7, 7, 7, 7, 7, 3, 2]
NCHUNK = len(CHUNKS)
BOFF = [0]
for _b in CHUNKS:
    BOFF.append(BOFF[-1] + _b)
assert BOFF[-1] == BLKS
MAXB = max(CHUNKS)              # 7
ROWS = MAXB * G                 # 112 gather columns (max, per partition)
IDXCOLS = BLKS * 2048 // 16     # 6272 idx-tile columns total

# dtype knobs ("f32" or "bf16").
GATHER_DT = os.environ.get("KERNEL_GATHER_DT", "bf16")
MLP_DT = os.environ.get("KERNEL_MLP_DT", "f32")
MC_BUFS = int(os.environ.get("KERNEL_MC_BUFS", "4"))
MX_BUFS = int(os.environ.get("KERNEL_MX_BUFS", "1"))
GSUB = int(os.environ.get("KERNEL_GSUB", "4"))    # sub-gathers per chunk
NQUEUES = int(os.environ.get("KERNEL_NQUEUES", "4"))  # SWDGE queues (Q7 pairs)

_DT = {"f32": mybir.dt.float32, "bf16": mybir.dt.bfloat16}

f32 = mybir.dt.float32
i16 = mybir.dt.int16


def _splits(nb):
    """MLP free-dim splits for a chunk of `nb` blocks (nb*128 columns)."""
    if nb * 128 <= 512:
        return [(0, nb * 128)]
    assert nb == 7
    return [(0, 448), (448, 448)]


# ---------------------------------------------------------------- device IR
def build_module():
    gdt = _DT[GATHER_DT]
    mdt = _DT[MLP_DT]

    nc = bacc.Bacc(
        "TRN2",
        target_bir_lowering=False,
        debug=False,
        enable_asserts=False,
        num_devices=NCORES,
        num_swdge_queues=NQUEUES,
    )

    tab = nc.dram_tensor("tab", [BLKS * 2048, D], gdt, kind="ExternalInput")
    idx = nc.dram_tensor("idx", [128, IDXCOLS], i16, kind="ExternalInput")
    w1 = nc.dram_tensor("w1", [128, 256], mdt, kind="ExternalInput")
    b1 = nc.dram_tensor("b1", [128, 2], f32, kind="ExternalInput")
    w2 = nc.dram_tensor("w2", [128, 256], mdt, kind="ExternalInput")
    b2 = nc.dram_tensor("b2", [128, 1], f32, kind="ExternalInput")
    w3 = nc.dram_tensor("w3", [128, 1], mdt, kind="ExternalInput")
    b3 = nc.dram_tensor("b3", [1, 1], f32, kind="ExternalInput")
    idn = nc.dram_tensor("ident", [128, 128], mdt, kind="ExternalInput")
    out = nc.dram_tensor("out", [HPAD], f32, kind="ExternalOutput")

    out2d = out.ap().rearrange("(a b) -> a b", a=1)  # [1, HPAD]

    with tile.TileContext(nc) as tc:
        with (
            tc.tile_pool(name="const", bufs=1) as cp,
            tc.tile_pool(name="mem", bufs=2) as mp,
            tc.tile_pool(name="scr", bufs=2) as sp,
            tc.tile_pool(name="mlp", bufs=2) as lp,
            tc.tile_pool(name="pst", bufs=2, space="PSUM") as pt,
            tc.tile_pool(name="psm", bufs=1, space="PSUM") as pm,
        ):
            ident = cp.tile([128, 128], mdt)
            w1_t = cp.tile([128, 256], mdt)
            b1_t = cp.tile([128, 2], f32)
            w2_t = cp.tile([128, 256], mdt)
            b2_t = cp.tile([128, 1], f32)
            w3_t = cp.tile([128, 1], mdt)
            b3_t = cp.tile([1, 1], f32)

            nc.sync.dma_start(out=ident[:], in_=idn.ap())
            nc.sync.dma_start(out=w1_t[:], in_=w1.ap())
            nc.sync.dma_start(out=b1_t[:], in_=b1.ap())
            nc.sync.dma_start(out=w2_t[:], in_=w2.ap())
            nc.sync.dma_start(out=b2_t[:], in_=b2.ap())
            nc.sync.dma_start(out=w3_t[:], in_=w3.ap())
            nc.sync.dma_start(out=b3_t[:], in_=b3.ap())

            for ch in range(NCHUNK):
                nb = CHUNKS[ch]
                memb = nb * 2048              # member slots this chunk
                trow0 = BOFF[ch] * 2048       # table row base
                icol0 = BOFF[ch] * 128        # idx-tile column base

                # ---- per-chunk idx load (small, ahead of the gather) ----
                ib = mp.tile([128, MAXB * 128], i16, tag="ib", bufs=3)
                nc.sync.dma_start(
                    out=ib[:, :memb // 16],
                    in_=idx.ap()[:, icol0:icol0 + memb // 16])

                # ---- gather the chunk with GSUB dma_gathers (4 queues) ----
                Mc = mp.tile([128, ROWS * D], gdt, tag="mc", bufs=MC_BUFS)
                M3 = Mc[:, :nb * G * D].rearrange("p (c d) -> p c d", d=D)
                ni = memb // GSUB             # idx per sub-gather
                nw = ni // 16                 # idx-tile columns per sub
                nd = ni // 128                # dest columns per sub
                for g in range(GSUB):
                    nc.gpsimd.dma_gather(
                        out_ap=M3[:, g * nd:(g + 1) * nd, :],
                        in_ap=tab.ap()[trow0:trow0 + memb, :],
                        idxs_ap=ib[:, g * nw:(g + 1) * nw],
                        num_idxs=ni,
                        num_idxs_reg=ni,
                        elem_size=D,
                        single_packet=False,
                        queue_num=g % NQUEUES,
                    )

                M4 = Mc[:, :nb * G * D].rearrange("p (b m d) -> p b m d",
                                                  m=G, d=D)

                # ---- segment max (tree into scratch, whole chunk) ----
                mx = sp.tile([128, MAXB * 8 * D], gdt, tag="mx", bufs=MX_BUFS)
                mx4 = mx[:, :nb * 8 * D].rearrange("p (b m d) -> p b m d",
                                                   m=8, d=D)
                nc.vector.tensor_tensor(
                    out=mx4, in0=M4[:, :, 0:8, :], in1=M4[:, :, 8:16, :],
                    op=mybir.AluOpType.max)
                nc.vector.tensor_tensor(
                    out=mx4[:, :, 0:4, :], in0=mx4[:, :, 0:4, :],
                    in1=mx4[:, :, 4:8, :], op=mybir.AluOpType.max)
                nc.vector.tensor_tensor(
                    out=mx4[:, :, 0:2, :], in0=mx4[:, :, 0:2, :],
                    in1=mx4[:, :, 2:4, :], op=mybir.AluOpType.max)
                nc.vector.tensor_tensor(
                    out=mx4[:, :, 0:1, :], in0=mx4[:, :, 0:1, :],
                    in1=mx4[:, :, 1:2, :], op=mybir.AluOpType.max)

                # ---- segment min (tree in place inside Mc) ----
                nc.vector.tensor_tensor(
                    out=M4[:, :, 0:8, :], in0=M4[:, :, 0:8, :],
                    in1=M4[:, :, 8:16, :], op=mybir.AluOpType.min)
                nc.vector.tensor_tensor(
                    out=M4[:, :, 0:4, :], in0=M4[:, :, 0:4, :],
                    in1=M4[:, :, 4:8, :], op=mybir.AluOpType.min)
                nc.vector.tensor_tensor(
                    out=M4[:, :, 0:2, :], in0=M4[:, :, 0:2, :],
                    in1=M4[:, :, 2:4, :], op=mybir.AluOpType.min)
                nc.vector.tensor_tensor(
                    out=M4[:, :, 0:1, :], in0=M4[:, :, 0:1, :],
                    in1=M4[:, :, 1:2, :], op=mybir.AluOpType.min)

                # ---- emb = max - min  (hedge-on-partition layout) ----
                emb = sp.tile([128, MAXB * D], mdt, tag="emb", bufs=2)
                emb4 = emb[:, :nb * D].rearrange("p (b m d) -> p b m d",
                                                 m=1, d=D)
                nc.vector.tensor_tensor(
                    out=emb4, in0=mx4[:, :, 0:1, :], in1=M4[:, :, 0:1, :],
                    op=mybir.AluOpType.subtract)

                # ---- transpose each 128x128 block into embT ----
                embT = lp.tile([128, MAXB * 128], mdt, tag="embT")
                for b in range(nb):
                    ptile = pt.tile([128, 128], mdt, tag="ptr")
                    nc.tensor.transpose(
                        out=ptile[:], in_=emb[:, b * D:(b + 1) * D],
                        identity=ident[:])
                    nc.scalar.copy(
                        out=embT[:, b * 128:(b + 1) * 128], in_=ptile[:])

                # ---- MLP on the chunk ----
                for (so, W) in _splits(nb):
                    ns = slice(so, so + W)
                    h1 = lp.tile([128, 2 * 512], mdt, tag="h1")
                    p2 = pm.tile([128, 512], f32, tag="p2")
                    for o in range(2):
                        p1 = pm.tile([128, 512], f32, tag=f"p1{o}")
                        nc.tensor.matmul(
                            out=p1[:, :W], lhsT=w1_t[:, o * 128:(o + 1) * 128],
                            rhs=embT[:, ns], start=True, stop=True)
                        nc.scalar.activation(
                            out=h1[:, o * 512:o * 512 + W], in_=p1[:, :W],
                            func=mybir.ActivationFunctionType.Relu,
                            bias=b1_t[:, o:o + 1])
                    nc.tensor.matmul(
                        out=p2[:, :W], lhsT=w2_t[:, 0:128], rhs=h1[:, 0:W],
                        start=True, stop=False)
                    nc.tensor.matmul(
                        out=p2[:, :W], lhsT=w2_t[:, 128:256],
                        rhs=h1[:, 512:512 + W], start=False, stop=True)
                    h2 = lp.tile([128, 512], mdt, tag="h2")
                    nc.scalar.activation(
                        out=h2[:, :W], in_=p2[:, :W],
                        func=mybir.ActivationFunctionType.Relu,
                        bias=b2_t[:, 0:1])
                    p3 = pm.tile([1, 512], f32, tag="p3")
                    nc.tensor.matmul(
                        out=p3[:, :W], lhsT=w3_t[:, 0:1], rhs=h2[:, :W],
                        start=True, stop=True)
                    osb = lp.tile([1, 512], f32, tag="osb")
                    nc.scalar.activation(
                        out=osb[:, :W], in_=p3[:, :W],
                        func=mybir.ActivationFunctionType.Sigmoid,
                        bias=b3_t[:, 0:1])
                    base = BOFF[ch] * 128 + so
                    nc.sync.dma_start(
                        out=out2d[0:1, base:base + W], in_=osb[:, :W])

    nc.compile()
    return nc


# ---------------------------------------------------------------- host prep
def _np_dt(name):
    if name == "f32":
        return np.float32
    import ml_dtypes
    return ml_dtypes.bfloat16


def prepare_in_maps(v_feat, W1, b1, W2, b2, W3, b3, node_ids):
    gnp = _np_dt(GATHER_DT)
    mnp = _np_dt(MLP_DT)

    vfeat_h = np.ascontiguousarray(np.asarray(v_feat, np.float32)).astype(gnp)
    w1_h = np.asarray(W1, np.float32).astype(mnp)                     # [128,256]
    b1_h = np.ascontiguousarray(np.asarray(b1, np.float32).reshape(2, 128).T)
    w2_h = np.concatenate(
        [np.asarray(W2, np.float32)[0:128, :], np.asarray(W2, np.float32)[128:256, :]],
        axis=1).astype(mnp)                                            # [128,256]
    b2_h = np.asarray(b2, np.float32).reshape(128, 1)
    w3_h = np.asarray(W3, np.float32).astype(mnp)                      # [128,1]
    b3_h = np.asarray(b3, np.float32).reshape(1, 1)

    nid = np.asarray(node_ids).astype(np.int64)                        # [E]

    in_maps = []
    for c in range(NCORES):
        tab_core = np.zeros((BLKS * 2048, D), gnp)
        idx_core = np.zeros((128, IDXCOLS), np.int16)
        for ch in range(NCHUNK):
            nb = CHUNKS[ch]
            memb = nb * 2048
            # hedge h_local = (BOFF[ch]+b)*128 + p (clamped), member m
            hl = ((BOFF[ch] + np.arange(nb))[:, None] * 128
                  + np.arange(128)[None, :])                           # [b,p]
            hglob = c * H_CORE + np.minimum(hl, H_CORE - 1)
            e = hglob[..., None] * G + np.arange(G)                    # [b,p,m]
            ids = nid[e]
            uniq, inv = np.unique(ids, return_inverse=True)
            tab_core[BOFF[ch] * 2048:BOFF[ch] * 2048 + len(uniq)] = vfeat_h[uniq]
            inv = inv.reshape(nb, 128, G).astype(np.int16)             # [b,p,m]
            # list position i = c_col*128 + p,  c_col = b*16 + m
            lst = inv.transpose(0, 2, 1).reshape(memb)                 # [(b,m),p]
            wrapped = lst.reshape(memb // 16, 16).T                    # [16, memb/16]
            idx_core[:, BOFF[ch] * 128:BOFF[ch] * 128 + memb // 16] = (
                np.tile(wrapped, (8, 1)))
        in_maps.append({
            "ident": np.eye(128, dtype=mnp),
            "tab": tab_core,
            "idx": idx_core,
            "w1": w1_h, "b1": b1_h,
            "w2": w2_h, "b2": b2_h,
            "w3": w3_h, "b3": b3_h,
        })
    return in_maps


def assemble_output(results):
    """results: list (per core) of {'out': [HPAD] f32} -> [NH, 1] f32."""
    outs = []
    for c in range(NCORES):
        o = np.asarray(results[c]["out"], np.float32).reshape(HPAD)
        outs.append(o[:H_CORE])
    return np.concatenate(outs).reshape(NH, 1)


# ---------------------------------------------------------------- entry
_CACHED_NC = None
LAST_RESULTS = None


def _ensure_ntff_hook():
    """The image's antenv lacks axon_hooks; if tracing is ever requested
    (e.g. BASS_TRACE in the environment), bass_utils would ImportError.
    Provide a stub so the run degrades gracefully instead of crashing."""
    import sys
    import types
    try:
        import antenv.axon_hooks  # noqa: F401
        return
    except ImportError:
        pass
    try:
        hook = None
        try:
            from trn_agent_boot.trn_boot import _ntff_profile_via_ctypes
            hook = _ntff_profile_via_ctypes("/opt/axon/libaxon_pjrt.so")
        except Exception:
            hook = None
        mod = types.ModuleType("antenv.axon_hooks")
        mod._hook = hook
        mod.get_axon_ntff_profile_hook = lambda: mod._hook
        mod.set_axon_ntff_profile_hook = lambda h: setattr(mod, "_hook", h)
        import antenv
        antenv.axon_hooks = mod
        sys.modules["antenv.axon_hooks"] = mod
    except Exception:
        pass


def _numpy_fallback(v_feat, W1, b1, W2, b2, W3, b3, node_ids, segment_ids):
    """General (slow, host) path for non-uniform segments; never taken for
    the reference's setup_inputs, which always emits repeat(arange(NH), 16)."""
    v = np.asarray(v_feat, np.float32)
    feats = v[np.asarray(node_ids).astype(np.int64)]
    seg = np.asarray(segment_ids).astype(np.int64)
    mx = np.full((NH, D), -np.inf, np.float32)
    mn = np.full((NH, D), np.inf, np.float32)
    np.maximum.at(mx, seg, feats)
    np.minimum.at(mn, seg, feats)
    emb = mx - mn
    h = np.maximum(emb @ np.asarray(W1, np.float32) + np.asarray(b1, np.float32), 0)
    h = np.maximum(h @ np.asarray(W2, np.float32) + np.asarray(b2, np.float32), 0)
    z = h @ np.asarray(W3, np.float32) + np.asarray(b3, np.float32)
    return (1.0 / (1.0 + np.exp(-z))).astype(np.float32)


def kernel(v_feat, W1, b1, W2, b2, W3, b3, node_ids, segment_ids):
    global _CACHED_NC, LAST_RESULTS

    seg = np.asarray(segment_ids)
    if seg.shape != (E,) or not np.array_equal(
            seg[::G], np.arange(NH, dtype=seg.dtype)) or not np.array_equal(
            seg, np.repeat(seg[::G], G)):
        return _numpy_fallback(v_feat, W1, b1, W2, b2, W3, b3,
                               node_ids, segment_ids)

    in_maps = prepare_in_maps(v_feat, W1, b1, W2, b2, W3, b3, node_ids)

    _ensure_ntff_hook()
    if _CACHED_NC is None:
        _CACHED_NC = build_module()
    nc = _CACHED_NC

    res = bass_utils.run_bass_kernel_spmd(
        nc, in_maps, core_ids=list(range(NCORES)))
    LAST_RESULTS = res
    return assemble_output(res.results)
